# revision 30
# baseline (speedup 1.0000x reference)
"""Linear-chain CRF loss (mean over batch of logZ - gold_score) on 8 TRN2 cores.

Math: the forward (alpha) recursion runs in the exp domain:
    a_t = ee_t * (E^T a_{t-1}),   E = exp(transitions - MU),  ee = exp(emissions)
logZ = log(eend^T a_{T-1}) + MU*(T-1).

Key restructure — segmented scan with warmup: products of positive matrices
forget their initial direction at the Birkhoff contraction rate, measured here
at ~0.15x per step (bf16 noise floor ~5e-3 Hilbert reached after ~6 steps).
The serial T-1 = 1023-step chain is split into S=32 segments; each segment's
start direction u_s is recovered by warming up W=6..7 steps from a uniform
vector using the true preceding emissions.  The exact telescoping identity
    logZ = sum_s log(1^T P_s u_s) - sum_{s>=1} log(1^T u_s)
(segment 0 starts from the exact a_0; exp(end) is folded into the last
segment's final emission slot) makes the answer independent of the u_s scale;
direction error enters once per boundary, ~1e-3 in log units, giving ~1e-5
relative error on the loss (tolerance 2e-2).

All 32 chains run concurrently in lockstep bundles of G=16 chains: per
iteration each bundle does ONE [128x128]@[128x256] matmul (PE) and ONE
[128,256] elementwise multiply (DVE), amortizing the fixed PSUM-access cost
over 256 columns.  This converts the latency-bound serial recursion
(~535ns/step) into a throughput-bound pipeline (~24ns/step/chain).

Host precomputes all constants in the exp domain (exp(trans-MU) bf16, the
exact a_0 = exp(em_0 + start), end folded into the last emission) so the
device prologue is just DMA -> exp(emissions) -> go.

Scheduling: raw Bass with manual semaphores (one wait per instruction, so
instructions pre-decode into engine wait queues instead of blocking
sequencers).  The per-step multiply must run on DVE: GPSIMD/Pool cannot
access PSUM, and the Activation engine only supports per-partition scalars.

Sharding: data-parallel over batch, 16 sequences per core, no collectives;
host computes the (tiny) gold path score, capture logs, and the final mean.
"""

import numpy as np
from contextlib import ExitStack

import ml_dtypes
import concourse.bass as bass
import concourse.bacc as bacc
import concourse.mybir as mybir
from concourse import bass_utils

B, T, C = 128, 1024, 128
NCORES = 8
BLOC = B // NCORES            # 16 sequences per core
S = 32                        # time segments (chains) per core
G = 16                        # chains per lockstep bundle
NB = S // G                   # bundles
CW = G * BLOC                 # bundle width (matmul/mul free dim) = 256
NL = 37                       # lockstep iterations per chain
# Segment lengths: chain 0 starts from the exact a_0 (no warmup); the rest
# warm up for NL - L_k steps (5 or 6).  sum(LS) = T-1 = 1023.
LS = [37] + [32] * 25 + [31] * 6
WS = [NL - L for L in LS]     # warmup iterations per chain
# Emission slot layout: slot 0 holds the initial state ([a_0 | ones]);
# iteration i consumes slot i+1.  Chunk sizes in slots; small leading chunks
# let the pipeline start as soon as the first slots land (emissions arrive
# pre-exponentiated from the host, so DMA completion is the only gate).
CS = [2, 2, 3, 4, 5, 5, 5, 5, 5, 2]
NCHUNK = len(CS)
_CHUNK_OF = []                # slot -> chunk index
_CHUNK_BASE = []              # chunk -> first slot
for _ci, _n in enumerate(CS):
    _CHUNK_BASE.append(len(_CHUNK_OF))
    _CHUNK_OF += [_ci] * _n
assert len(_CHUNK_OF) == NL + 1
MU = 5.9                      # per-step log-growth pre-subtraction
RA = 4                        # state-tile ring depth per bundle
NP = 3                        # PSUM-tile ring depth per bundle

F32 = mybir.dt.float32
BF16 = mybir.dt.bfloat16
AF = mybir.ActivationFunctionType

_cache = {}


def _plan():
    """ts[k][i] = original timestep whose emission iteration i applies for
    chain k (warmup uses the true preceding emissions)."""
    bs = np.concatenate([[1], 1 + np.cumsum(LS)])[:S]  # segment starts
    ts = np.zeros((S, NL), dtype=np.int64)
    for k in range(S):
        ts[k, :] = (bs[k] - WS[k]) + np.arange(NL)
    return ts


def _build():
    if "nc" in _cache:
        return _cache["nc"]
    nc = bacc.Bacc("TRN2", target_bir_lowering=False, debug=False)
    em = nc.dram_tensor("em", (C, NL + 1, S * BLOC), BF16, kind="ExternalInput")
    ef = nc.dram_tensor("ef", (C, C), BF16, kind="ExternalInput")
    caps = nc.dram_tensor("caps", (2, S * BLOC), F32, kind="ExternalOutput")

    with ExitStack() as ctx:
        _n = iter(range(10 ** 6))
        sbuf = lambda shape, dt: ctx.enter_context(
            nc.sbuf_tensor(f"sb{next(_n)}", shape, dt))
        psum = lambda shape, dt: ctx.enter_context(
            nc.psum_tensor(f"ps{next(_n)}", shape, dt))

        s_dma_c = nc.alloc_semaphore("s_dma_c")   # ef DMA (+16)
        s_dma_e = nc.alloc_semaphore("s_dma_e")   # emission chunk DMAs
        s_pe = [nc.alloc_semaphore(f"s_pe{g}") for g in range(NB)]
        s_po = [nc.alloc_semaphore(f"s_po{g}") for g in range(NB)]
        s_cap = nc.alloc_semaphore("s_cap")       # capture matmuls
        s_tail = nc.alloc_semaphore("s_tail")

        Ef = sbuf([C, C], BF16)
        ones_col = sbuf([C, 1], BF16)
        warm = sbuf([1, 1], F32)
        ee = [sbuf([C, CS[ci], S * BLOC], BF16) for ci in range(NCHUNK)]
        st = [[sbuf([C, CW], BF16) for _ in range(RA)] for _ in range(NB)]
        sc_sb = sbuf([1, S * BLOC], F32)
        ec_sb = sbuf([1, S * BLOC], F32)

        ps = [[psum([C, CW], F32) for _ in range(NP)] for _ in range(NB)]
        scap = psum([1, S * BLOC], F32)
        ecap = psum([1, S * BLOC], F32)

        def ee_sl(g, slot):
            ci = _CHUNK_OF[slot]
            return ee[ci][:, slot - _CHUNK_BASE[ci], g * CW:(g + 1) * CW]

        # x_0 is the init slot of the ee stream itself: [a_0 | ones...]
        def x0_sl(g):
            return ee[0][:, 0, g * CW:(g + 1) * CW]

        # --- SP queue: first chunk (holds x_0) leads, then Ef, then chunks ---
        nc.sync.dma_start(
            out=ee[0][:], in_=em[:, 0:CS[0], :]).then_inc(s_dma_e, 16)
        nc.sync.dma_start(out=Ef[:, :], in_=ef[:, :]).then_inc(s_dma_c, 16)
        for ci in range(1, NCHUNK):
            base = _CHUNK_BASE[ci]
            nc.sync.dma_start(
                out=ee[ci][:], in_=em[:, base:base + CS[ci], :]
            ).then_inc(s_dma_e, 16)

        # --- Act queue: dummy copy so its func table loads off the tail path
        nc.scalar.activation(warm[:], warm[:], AF.Copy)

        # --- DVE queue: ones column for the capture matmuls ---
        s_one = nc.alloc_semaphore("s_one")
        nc.vector.memset(ones_col[:], 1.0).then_inc(s_one, 1)

        # --- PE prologue: p-state warmers (results unread), then input gates
        for w in range(12):
            nc.tensor.matmul(
                ps[0][w % NP][:], st[1][1][:, 0:C], st[1][1][:],
                start=True, stop=True)
        nc.tensor.wait_ge(s_dma_c, 16)   # Ef
        nc.tensor.wait_ge(s_one, 1)      # ones_col

        def runs_of(ks):
            """Group chain ids into (bundle, c_lo, c_hi) runs of adjacent
            columns so each capture is one wide matmul."""
            out = []
            for k in sorted(ks):
                g, c = divmod(k, G)
                if out and out[-1][0] == g and out[-1][2] == c - 1:
                    out[-1][2] = c
                else:
                    out.append([g, c, c])
            return out

        cap_at = {}
        for k in range(S):
            cap_at.setdefault(WS[k], []).append(k)
        ncap = 0

        # --- main loop: NL lockstep iterations over NB bundles ---
        # x_0 is the ee stream's init slot; x_i (i>=1) lives in st[g][i % RA],
        # written by TT(i-1).  Iteration i applies emission slot i+1.
        #   mm(g,i)  waits s_po[g] >= i  (i=0: first-chunk DMA), incs s_pe[g]
        #   TT(g,i)  waits s_pe[g] >= i+1, incs s_po[g]
        for i in range(NL):
            slot = i + 1
            if _CHUNK_OF[slot] != _CHUNK_OF[slot - 1]:
                nc.vector.wait_ge(s_dma_e, 16 * (_CHUNK_OF[slot] + 1))
            for g in range(NB):
                rhs = x0_sl(g) if i == 0 else st[g][i % RA][:]
                mm = nc.tensor.matmul(
                    ps[g][i % NP][:], Ef[:], rhs, start=True, stop=True)
                if i == 0:
                    mm._wait_ge(s_dma_e, 16)
                else:
                    mm._wait_ge(s_po[g], i)
                mm.then_inc(s_pe[g], 1)
            # start-captures: 1^T u_k for chains whose warmup ends here
            for g, c0, c1 in runs_of(cap_at.get(i, [])):
                rhs = (x0_sl(g) if i == 0 else st[g][i % RA][:])[
                    :, c0 * BLOC:(c1 + 1) * BLOC]
                mm = nc.tensor.matmul(
                    scap[:, (g * G + c0) * BLOC:(g * G + c1 + 1) * BLOC],
                    ones_col[:], rhs, start=True, stop=True)
                mm._wait_ge(s_dma_e, 16) if i == 0 else mm._wait_ge(s_po[g], i)
                mm.then_inc(s_cap, 1)
                ncap += 1
            for g in range(NB):
                nc.vector.tensor_mul(
                    st[g][(i + 1) % RA][:], ps[g][i % NP][:], ee_sl(g, slot)
                )._wait_ge(s_pe[g], i + 1).then_inc(s_po[g], 1)

        nscap = ncap  # start-captures emitted so far

        # --- end-captures: 1^T x_NL per chain (end folded into emissions) ---
        for g in range(NB):
            nc.tensor.matmul(
                ecap[:, g * CW:(g + 1) * CW], ones_col[:],
                st[g][NL % RA][:], start=True, stop=True,
            )._wait_ge(s_po[g], NL).then_inc(s_cap, 1)
            ncap += 1

        # --- tail: copy captures PSUM->SBUF on Act (DMA can't read PSUM);
        # start-captures copied as soon as they're all done (mid-loop) ---
        nc.scalar.activation(
            sc_sb[:], scap[:], AF.Copy)._wait_ge(s_cap, nscap).then_inc(s_tail, 1)
        nc.scalar.activation(
            ec_sb[:], ecap[:], AF.Copy)._wait_ge(s_cap, ncap).then_inc(s_tail, 1)
        s_out = nc.alloc_semaphore("s_out")
        nc.sync.wait_ge(s_tail, 1)
        nc.sync.dma_start(out=caps[0:1, :], in_=sc_sb[:]).then_inc(s_out, 16)
        nc.sync.wait_ge(s_tail, 2)
        nc.sync.dma_start(out=caps[1:2, :], in_=ec_sb[:]).then_inc(s_out, 16)

    nc.compile()
    _cache["nc"] = nc
    return nc


def _gold_np(emissions, tags, mask, transitions, start_transitions, end_transitions):
    em = emissions.astype(np.float64)
    mf = mask.astype(np.float64)
    idx = np.arange(B)
    emit = np.take_along_axis(em, tags[:, :, None], axis=2)[:, :, 0]
    tr = transitions.astype(np.float64)[tags[:, :-1], tags[:, 1:]]
    score = start_transitions.astype(np.float64)[tags[:, 0]] + emit[:, 0]
    score = score + np.sum((emit[:, 1:] + tr) * mf[:, 1:], axis=1)
    last_idx = mask.astype(np.int64).sum(axis=1) - 1
    last_tags = tags[idx, last_idx]
    return score + end_transitions.astype(np.float64)[last_tags]


def _logz_host(emissions, mask, transitions, start_transitions, end_transitions):
    # Slow exact fallback (only for non-all-ones masks, which the spec never
    # produces).
    em = emissions.astype(np.float64)
    tr = transitions.astype(np.float64)
    alpha = start_transitions.astype(np.float64) + em[:, 0]
    for t in range(1, T):
        sc = alpha[:, :, None] + tr[None] + em[:, t, None, :]
        m = sc.max(axis=1)
        nxt = m + np.log(np.exp(sc - m[:, None, :]).sum(axis=1))
        alpha = np.where(mask[:, t, None], nxt, alpha)
    fin = alpha + end_transitions.astype(np.float64)[None]
    m = fin.max(axis=1)
    return m + np.log(np.exp(fin - m[:, None]).sum(axis=1))


def run_device(in_maps, trace=False, **kw):
    nc = _build()
    return bass_utils.run_bass_kernel_spmd(
        nc, in_maps, core_ids=list(range(NCORES)), trace=trace, **kw)


def make_in_maps(emissions, transitions, start_transitions, end_transitions):
    ts = _plan()
    ef = np.exp(transitions.astype(np.float64) - MU).astype(ml_dtypes.bfloat16)
    ef = np.ascontiguousarray(ef)
    in_maps = []
    for k in range(NCORES):
        sl = slice(k * BLOC, (k + 1) * BLOC)
        em_k = emissions[sl].transpose(2, 1, 0).astype(np.float64)  # (C,T,BLOC)
        # (C, S, NL, BLOC): per-chain emission streams (warmup + segment)
        em_g = em_k[:, ts, :]
        # fold exp(end) into the last chain's final slot
        em_g[:, S - 1, NL - 1, :] += end_transitions.astype(np.float64)[:, None]
        ee_g = np.empty((C, NL + 1, S, BLOC), dtype=np.float64)
        ee_g[:, 1:] = np.exp(em_g).transpose(0, 2, 1, 3)
        # slot 0 = initial state: exact a_0 for chain 0, uniform elsewhere
        ee_g[:, 0] = 1.0
        ee_g[:, 0, 0] = np.exp(
            em_k[:, 0, :] + start_transitions.astype(np.float64)[:, None])
        ee_g = ee_g.reshape(C, NL + 1, S * BLOC)
        in_maps.append({
            "em": np.ascontiguousarray(ee_g.astype(ml_dtypes.bfloat16)),
            "ef": ef,
        })
    return in_maps


def kernel(**inputs):
    emissions = np.asarray(inputs["emissions"], dtype=np.float32)
    tags = np.asarray(inputs["tags"]).astype(np.int64)
    mask = np.asarray(inputs["mask"]).astype(bool)
    transitions = np.asarray(inputs["transitions"], dtype=np.float32)
    start_transitions = np.asarray(inputs["start_transitions"], dtype=np.float32)
    end_transitions = np.asarray(inputs["end_transitions"], dtype=np.float32)

    gold = _gold_np(emissions, tags, mask, transitions,
                    start_transitions, end_transitions)

    if mask.all():
        in_maps = make_in_maps(emissions, transitions,
                               start_transitions, end_transitions)
        res = run_device(in_maps)
        logz = np.empty(B, dtype=np.float64)
        for k in range(NCORES):
            caps = np.asarray(res.results[k]["caps"], dtype=np.float64)
            sc = caps[0].reshape(S, BLOC)
            ec = caps[1].reshape(S, BLOC)
            # telescoped segment growths; E carries exp(-MU) on each of the
            # 1023 real steps
            lz = np.log(ec).sum(0) - np.log(sc[1:]).sum(0) + MU * (T - 1)
            logz[k * BLOC:(k + 1) * BLOC] = lz
    else:
        logz = _logz_host(emissions, mask, transitions,
                          start_transitions, end_transitions)

    loss = np.mean(logz - gold)
    return np.asarray(loss, dtype=np.float32)
